# revision 1
# baseline (speedup 1.0000x reference)
"""Trainium2 Bass kernel for nn_AttentionBlock (B=8, C=512, H=W=32, 8 heads).

Sharding: data-parallel over batch — core b computes batch image b end-to-end
(attention is independent per (batch, head), so this is embarrassingly
parallel; weights are replicated to all 8 cores).

Per-core pipeline (x_b viewed as (C=512, S=1024) channels-on-partition):
  P1a: q,k = Wqk^T.T @ x          -> (1024, S) PSUM->SBUF, channel order
       arranged (on host) so each 128-row tile is one head-PAIR of q or k.
  P1b: vT  = x.T @ Wv^T           -> (S, 512) directly transposed, so no PE
       transposes are needed for attention; a ones column is appended per
       head (65 cols/head) to produce softmax denominators for free.
  P2 : scoresT[t,s] = k^T q per head; head pairs run CONCURRENTLY in the PE
       array via row tiling (K=64 each at base partitions 0/64).
  exp: ACT exp(0.125 * scoresT) PSUM->SBUF, one (128,2048) instr per t-tile.
  P3 : outT_aug[h] = [vT|1]^T @ expT  (65, S): row 64 = softmax denominator.
  norm: DVE reciprocal -> ones outer-product PE broadcast -> DVE multiply.
  P4 : y = Wo^T.T @ res + bo + x (4 concurrent PSUM accumulators) -> DMA out.

All matmuls run as float32r (fp32 bits, full-rate 1 cycle/row PE mode).
next-pair q/k projections and pair-0 vT tiles are interleaved into the
attention j-loop so the PE fills the ACT (exp) slack; tiny "corner" matmuls
and scratch copies act as semaphore-wait carriers because several walrus
instruction structs encode only a single wait (see pe_mm/dve_sync and
_strip_self_waits/_install_drain_split).
"""

import os
import sys

for _p in ("/opt/trn_rl_repo", "/root/.axon_site/_ro/trn_rl_repo"):
    if os.path.isdir(_p) and _p not in sys.path:
        sys.path.insert(0, _p)

from contextlib import ExitStack

import numpy as np

import concourse.bass as bass
import concourse.tile as tile
from concourse import mybir
from concourse.bass_utils import run_bass_kernel_spmd

B, C, H, W = 8, 512, 32, 32
NH, D = 8, 64
S = H * W            # 1024 sequence positions
P = 128              # partitions
KT = C // P          # 4 contraction tiles over channels
MT_QK = 2 * C // P   # 8 output tiles for q,k
NT = S // P          # 8 t-tiles
NPAIR = NH // 2      # 4 head pairs
DA = D + 1           # 65: v columns + ones column per head
F32 = mybir.dt.float32
AF = mybir.ActivationFunctionType
ALU = mybir.AluOpType

EXP_BUFS = int(os.environ.get("K_EXP_BUFS", "12"))
ACT_K = int(os.environ.get("K_ACT_K", "8"))
INS_J = tuple(int(c) for c in os.environ.get("K_INS_J", "0134"))
USE_F32R = os.environ.get("K_F32R", "1") == "1"


def _r(ap):
    """Matmul-operand dtype: float32r streams 1 col/cycle (vs 4 for fp32)."""
    return ap.bitcast(mybir.dt.float32r) if USE_F32R else ap


def _install_drain_split():
    """walrus's CTRL_NO (drain) codegen accepts only a single semaphore wait,
    but Tile's kernel-tail drain aggregates one wait per live proc.  Split
    them across several serial drains (semantically identical: all complete
    before the closing all-engine barrier)."""
    if getattr(tile.TileContext, "_drain_split_installed", False):
        return
    from concourse.vector_clock import ScopedClock

    orig = tile.TileContext._drain_and_barrier

    def patched(self, tick_clock, wait_clock):
        nc = self.nc
        drain_inst = nc.sync.drain()
        wait_clock.add_sem_waits(
            drain_inst.ins, ScopedClock({None: tick_clock.global_clock})
        )
        si = drain_inst.ins.sync_info
        if si is not None and si.on_wait and len(si.on_wait) > 1:
            waits = list(si.on_wait)
            drain_inst.ins.sync_info = mybir.SyncInfo(
                on_wait=[waits[0]], on_update=list(si.on_update or [])
            )
            for w in waits[1:]:
                d2 = nc.sync.drain()
                d2.ins.sync_info = mybir.SyncInfo(on_wait=[w], on_update=[])

        nc.all_engine_barrier()
        assert self.sems is not None
        popped = nc._tile_sem_poison_stack.pop()
        assert popped is self._sem_poison
        nc.clear_and_free_semaphores(list(self.sems.allocated().values()))
        nc.all_engine_barrier()

    tile.TileContext._drain_and_barrier = patched
    tile.TileContext._drain_split_installed = True
    tile.TileContext._drain_and_barrier_orig = orig


def trace_kernel(ctx, tc, nc, x, wqk, y):
    cst = ctx.enter_context(tc.tile_pool(name="cst", bufs=1))
    qkp = ctx.enter_context(tc.tile_pool(name="qkp", bufs=4))
    expp = ctx.enter_context(tc.tile_pool(name="expp", bufs=EXP_BUFS))
    resp = ctx.enter_context(tc.tile_pool(name="resp", bufs=1))
    rdp = ctx.enter_context(tc.tile_pool(name="rdp", bufs=2))
    rbp = ctx.enter_context(tc.tile_pool(name="rbp", bufs=2))
    yp = ctx.enter_context(tc.tile_pool(name="yp", bufs=1))
    pa = ctx.enter_context(tc.tile_pool(name="pa", bufs=2, space="PSUM"))
    pb = ctx.enter_context(tc.tile_pool(name="pb", bufs=2, space="PSUM"))

    xt = cst.tile([P, KT, S], F32)
    wall = cst.tile([P, KT, 2 * C + C + C + 1], F32)
    wqkt = wall[:, :, 0:2 * C]
    wvt = wall[:, :, 2 * C:2 * C + C]
    wot = wall[:, :, 3 * C:4 * C]
    ones = cst.tile([1, D], F32)
    scr = cst.tile([1, 256], F32)
    scra = cst.tile([1, 8], F32)
    vta = cst.tile([P, NT, NH * DA], F32)
    res = resp.tile([P, KT, S], F32)

    nc.sync.dma_start(out=_r(xt[:, :, :]),
                      in_=_r(x.rearrange("(k p) s -> p k s", p=P)))
    wallr = wqk.rearrange("(k p) s -> p k s", p=P)
    nc.gpsimd.dma_start(out=_r(wall[:, :, 0:256]), in_=_r(wallr[:, :, 0:256]))
    nc.gpsimd.dma_start(out=_r(wall[:, :, 256:2 * C]),
                      in_=_r(wallr[:, :, 256:2 * C]))
    nc.gpsimd.dma_start(out=_r(wall[:, :, 2 * C:]), in_=_r(wallr[:, :, 2 * C:]))

    scr_i = [0]

    def dve_sync(*aps):
        # DVE wait-carrier: absorb one cross-engine wait per tiny copy.
        # Disjoint scratch columns avoid WAW self-waits between carriers.
        for ap in aps:
            n = ap.free_size()
            o = (scr_i[0] % 30) * 8
            scr_i[0] += 1
            nc.vector.tensor_copy(scr[0:1, o:o + n], ap)
    def pe_mm(corner, dep):
        # PE wait-carrier: a 1x2 matmul reading `dep` absorbs one cross-
        # engine wait; PE program order subsumes the tick for later matmuls.
        # `corner` is a PSUM slice overwritten by the next start=True group.
        nc.tensor.matmul(
            corner, _r(dep[:, 0:1]), _r(dep[:, 0:2]),
            start=True, stop=True, skip_group_check=True,
        )

    # exp(0*x) = 1.0 writes: DVE memset can't emit float32r, ACT can
    nc.scalar.activation(_r(ones[:, :]), _r(wall[0:1, 0, 0:D]), AF.Exp, scale=0.0)
    # ones column per head in the augmented vT (softmax denominator trick)
    nc.scalar.activation(
        _r(vta.rearrange("p j (h e) -> p j h e", h=NH)[:, :, :, D:DA]),
        _r(xt[:, 0, 0:NT * NH].rearrange("p (j h) -> p j h", h=NH)[:, :, :, None]
           if False else xt[:, 0, 0:NT * NH]),
        AF.Exp, scale=0.0,
    )

    dve_sync(xt[0:1, 0, 0:4])

    # PSUM: pa's single slot (128,2048) holds score tiles; pb's two (*,1024)
    # slots rotate between P1/P4 accumulators and P3 head accumulators.
    def acc_tile(i, shape):
        return pb.tile(shape, F32, tag="ob", name=f"acc{i}")

    qk_tiles = [None] * NPAIR
    nacc = 0
    ets_hist = []

    def act_sync_maybe():
        # Batched ACT wait-carrier: exp tiles cycle through EXP_BUFS slots;
        # each reuse makes the next exp wait on the slot's previous ACT
        # writer.  One cheap ACT copy pre-waiting on a newer tick covers the
        # next ACT_K reuses (the ACT semaphore is monotonic).
        n = len(ets_hist)
        if n >= EXP_BUFS and (n - EXP_BUFS) % ACT_K == 0:
            nc.scalar.copy(scra[0:1, 0:2], ets_hist[n - EXP_BUFS + ACT_K][0:1, 0:2])

    def p1a_mtile(m):
        nonlocal nacc
        pair, isk = divmod(m, 2)
        if isk == 0:
            qk_tiles[pair] = qkp.tile([P, 2 * S], F32, tag="qk", name=f"qk{pair}")
        acc = acc_tile(nacc, [P, S])
        nacc += 1
        if m == 0:
            pe_mm(acc[0:1, 0:2], wall[0:1, 0, 0:2])
        for n in range(2):
            for k in range(KT):
                nc.tensor.matmul(
                    acc[:, n * 512:(n + 1) * 512],
                    _r(wqkt[:, k, m * P:(m + 1) * P]),
                    _r(xt[:, k, n * 512:(n + 1) * 512]),
                    start=(k == 0),
                    stop=(k == KT - 1),
                )
        dve_sync(acc[0:1, 508:516])
        nc.vector.tensor_copy(
            _r(qk_tiles[pair][:, isk * S:(isk + 1) * S]), _r(acc[:, :])
        )

    def p1a_half(m, n, sync_ap=None):
        # half an m-tile (one 512-column n-slice) through a pa slot: small
        # enough to hide inside the attention j-loop's ACT slack
        pair, isk = divmod(m, 2)
        if isk == 0 and n == 0:
            qk_tiles[pair] = qkp.tile([P, 2 * S], F32, tag="qk", name=f"qk{pair}")
        acc = pa.tile([P, 512], F32, tag="sc", name=f"acc{m}_{n}")
        if m == 2 and n == 0 and sync_ap is not None:
            pe_mm(acc[0:1, 0:2], sync_ap)
            pe_mm(acc[0:1, 0:2], wall[0:1, 0, 256:258])
        if m == 0 and n == 0:
            pe_mm(acc[0:1, 0:2], wall[0:1, 0, 0:2])
        for k in range(KT):
            nc.tensor.matmul(
                acc[:, :],
                _r(wqkt[:, k, m * P:(m + 1) * P]),
                _r(xt[:, k, n * 512:(n + 1) * 512]),
                start=(k == 0),
                stop=(k == KT - 1),
            )
        dve_sync(acc[0:1, 252:260])
        nc.vector.tensor_copy(
            _r(qk_tiles[pair][:, isk * S + n * 512: isk * S + (n + 1) * 512]),
            _r(acc[:, :]),
        )

    def p1b_jtile(j, ets=None):
        # Lives in the pa pool: inside pair 0's j-loop both pb slots are
        # held by the oa accumulators.
        acc = pa.tile([P, C], F32, tag="sc", name=f"vacc{j}")
        if j == 0:
            pe_mm(acc[0:1, 0:2], ets[0][0:1, 0:2])
            pe_mm(acc[0:1, 0:2], wall[0:1, 0, 2 * C:2 * C + 2])
        for k in range(KT):
            nc.tensor.matmul(
                acc[:, :],
                _r(xt[:, k, j * P:(j + 1) * P]),
                _r(wvt[:, k, :]),
                start=(k == 0),
                stop=(k == KT - 1),
            )
        nc.vector.tensor_copy(
            _r(vta[:, j, :].rearrange("p (h e) -> p h e", h=NH)[:, :, 0:D]),
            _r(acc.rearrange("p (h d) -> p h d", h=NH)),
        )
        return acc

    def fused_pair(pair, norm_prev=None):
        """Per t-tile: scoresT matmuls -> exp -> attn@v accumulate.

        The P3 accumulation for t-tile j consumes exp tile j right away, so
        only EXP_BUFS exp tiles are ever live.  Pair 0 additionally computes
        vT (p1b) tile j inside round j — P3 only needs vta[:, j, :].
        """
        qk = qk_tiles[pair]
        oa = None
        for j in range(NT):
            ets, scs = [], []
            for hh in range(2):
                act_sync_maybe()
                et = expp.tile([P, S], F32, tag="et", name=f"et{pair}_{j}_{hh}")
                ets_hist.append(et)
                sc = pa.tile([P, S], F32, tag="sc", name=f"sc{pair}_{j}_{hh}")
                scs.append(sc)
                for n in range(2):
                    nc.tensor.matmul(
                        sc[:, n * 512:(n + 1) * 512],
                        _r(qk[64 * hh:64 * (hh + 1), S + j * P: S + (j + 1) * P]),
                        _r(qk[64 * hh:64 * (hh + 1), n * 512:(n + 1) * 512]),
                        start=True,
                        stop=True,
                    )
                nc.scalar.activation(
                    _r(et[:, :]), _r(sc[:, :]), AF.Exp, scale=1.0 / np.sqrt(D)
                )
                ets.append(et)
            if j == 0:
                if norm_prev is not None:
                    norm_prev()
                oa = [
                    pb.tile([DA, S], F32, tag="ob", name=f"oa{pair}_{hh}")
                    for hh in range(2)
                ]
                if pair == 0:
                    pe_mm(oa[0][0:1, 0:2], qk[0:1, S:S + 2])
                else:
                    pe_mm(oa[0][0:1, 0:2], res[64:65, pair - 1, 0:2])
            if pair == 0:
                vacc = p1b_jtile(j, ets)
                # DVE tick (vta j) rides on the dead vacc corner: its WAR
                # against the vacc evict is on the same DVE semaphore
                pe_mm(vacc[0:1, 0:2], vta[0:1, j, 0:2])
            ins_j = (4, 5, 6, 7) if pair == 0 else INS_J
            if pair < NPAIR - 1 and j in ins_j:
                mm_ = 2 * (pair + 1) + (j >= ins_j[2])
                p1a_half(mm_, 0 if j in (ins_j[0], ins_j[2]) else 1,
                         sync_ap=ets[0][0:1, 0:2])
            for hh in range(2):
                h = 2 * pair + hh
                for n in range(2):
                    nc.tensor.matmul(
                        oa[hh][:, n * 512:(n + 1) * 512],
                        _r(vta[:, j, h * DA:(h + 1) * DA]),
                        _r(ets[hh][:, n * 512:(n + 1) * 512]),
                        start=(j == 0),
                        stop=(j == NT - 1),
                        skip_group_check=True,
                    )
        def do_norm():
            for hh in range(2):
                rd = rdp.tile([1, S], F32, tag="rd", name=f"rd{pair}_{hh}")
                with nc.allow_low_precision(reason="f32r view of reciprocal"):
                    nc.vector.reciprocal(_r(rd[:, :]), oa[hh][D:DA, :])
                # broadcast 1/denom across the 64 head channels: ones (1,64)
                # outer-product matmul, then evict and multiply on DVE
                bc = pa.tile([D, S], F32, tag="sc", name=f"bc{pair}_{hh}")
                pe_mm(bc[0:1, 0:2], ets_hist[-2 + hh][0:1, 0:2])
                pe_mm(bc[0:1, 0:2], rd[0:1, 0:2])
                for n in range(2):
                    nc.tensor.matmul(
                        bc[:, n * 512:(n + 1) * 512],
                        _r(ones[:, :]),
                        _r(rd[:, n * 512:(n + 1) * 512]),
                        start=True,
                        stop=True,
                    )
                rb = rbp.tile([D, S], F32, tag="rb", name=f"rb{pair}_{hh}")
                nc.vector.tensor_copy(rb[:, :], bc[:, :])
                nc.vector.tensor_mul(
                    _r(res[64 * hh:64 * (hh + 1), pair, :]),
                    _r(oa[hh][0:D, :]), _r(rb[:, :]),
                )
        return do_norm

    # ---- schedule trace ----
    p1a_half(0, 0)
    p1a_half(1, 0)
    p1a_half(0, 1)
    p1a_half(1, 1)
    norm_prev = None
    for pair in range(NPAIR):
        norm_prev = fused_pair(pair, norm_prev)
    norm_prev()

    dve_sync(xt[0:1, 0, 4:8], wall[0:1, 0, 4 * C:4 * C + 1])
    ybig = yp.tile([P, KT, S], F32, tag="y", name="yb")
    for m in range(KT):
        if m >= 2:
            acc = pa.tile([P, S], F32, tag="sc", name=f"p4acc{m}")
        else:
            acc = acc_tile(m, [P, S])
        if m == 0:
            pe_mm(acc[0:1, 0:2], res[64:65, NPAIR - 1, 0:2])
        elif m == 1:
            pe_mm(acc[0:1, 0:2], res[0:1, NPAIR - 1, 0:2])
        for n in range(2):
            for k in range(KT):
                nc.tensor.matmul(
                    acc[:, n * 512:(n + 1) * 512],
                    _r(wot[:, k, m * P:(m + 1) * P]),
                    _r(res[:, k, n * 512:(n + 1) * 512]),
                    start=(k == 0),
                    stop=(k == KT - 1),
                )
        dve_sync(acc[0:1, 508:516])
        nc.vector.scalar_tensor_tensor(
            _r(ybig[:, m, :]), acc[:, :], wall[:, m, 4 * C:4 * C + 1],
            xt[:, m, :], op0=ALU.add, op1=ALU.add,
        )
        if m == 1 or m == KT - 1:
            yr = y.rearrange("(k p) s -> p k s", p=P)
            nc.gpsimd.dma_start(
                out=yr[:, m - 1:m + 1, :], in_=ybig[:, m - 1:m + 1, :]
            )


ENGINE_SEM_PREFIX = {
    "PE": "PE_",
    "Activation": "Activation_",
    "DVE": "DVE_",
    "Pool": "Pool_",
    "SP": "SP_",
}


def _strip_self_waits(nc):
    """Drop same-engine semaphore self-waits from multi-wait instructions.

    Engines execute and complete their own instructions in program order
    (PE matmuls are pc-monotone in start and end; ACT/DVE/Pool are strict
    FIFO with per-op drains), so a wait on the engine's own completion
    semaphore is redundant whenever the instruction carries another wait —
    and walrus's PE/ACT instruction structs only encode a single wait.
    """
    n = 0
    for inst in nc.inst_map.values():
        si = getattr(inst, "sync_info", None)
        if si is None or not si.on_wait or len(si.on_wait) <= 1:
            continue
        eng = str(getattr(inst, "engine", "")).split(".")[-1]
        pref = ENGINE_SEM_PREFIX.get(eng)
        if pref is None:
            continue
        keep = [w for w in si.on_wait if not w.ant_name.startswith(pref)]
        if len(keep) != len(si.on_wait) and keep:
            inst.sync_info = mybir.SyncInfo(
                on_wait=keep, on_update=list(si.on_update or [])
            )
            n += 1
    return n


def build_nc():
    _install_drain_split()
    nc = bass.Bass(trn_type="TRN2", debug=False, num_devices=8)
    x_d = nc.dram_tensor("x", [C, S], F32, kind="ExternalInput")
    wqk_d = nc.dram_tensor("wqkt", [C, 4 * C + 1], F32, kind="ExternalInput")
    y_d = nc.dram_tensor("y", [C, S], F32, kind="ExternalOutput")
    with tile.TileContext(nc) as tc, ExitStack() as ctx:
        trace_kernel(ctx, tc, nc, x_d.ap(), wqk_d.ap(), y_d.ap())
    _strip_self_waits(nc)
    if not nc.is_finalized():
        nc.finalize()
    return nc


def host_inputs(x, Wqkv, Wo, bo):
    """Host-side reshard: per-core input dicts (weights replicated)."""
    x = np.ascontiguousarray(np.asarray(x, dtype=np.float32))
    Wqkv = np.asarray(Wqkv, dtype=np.float32)
    Wo = np.asarray(Wo, dtype=np.float32)
    bo = np.asarray(bo, dtype=np.float32)

    # Wqkv rows per head h: [h*3D, h*3D+D) = q, [+D, +2D) = k, [+2D, +3D) = v.
    # q,k channel order: per pair -> [q(2p)|q(2p+1)], [k(2p)|k(2p+1)] tiles.
    order = []
    for p in range(NPAIR):
        for h in (2 * p, 2 * p + 1):
            order.extend(range(h * 3 * D, h * 3 * D + D))          # q rows
        for h in (2 * p, 2 * p + 1):
            order.extend(range(h * 3 * D + D, h * 3 * D + 2 * D))  # k rows
    wqkt = Wqkv[order].T                                            # (C, 2C)
    v_order = [h * 3 * D + 2 * D + d for h in range(NH) for d in range(D)]
    wvt = Wqkv[v_order].T                                           # (C, C)
    wot = Wo.T                                                      # (C, C)
    wall = np.ascontiguousarray(
        np.concatenate([wqkt, wvt, wot, bo[:, None]], axis=1)
    )                                                               # (C, 4C+1)

    return [
        dict(x=np.ascontiguousarray(x[b].reshape(C, S)), wqkt=wall)
        for b in range(B)
    ]


_NC_CACHE = []

try:
    # bass_exec HLO does not embed the BIR; bust jax's executable cache so a
    # rebuilt kernel is actually recompiled instead of hitting a stale NEFF.
    import jax as _jax

    _jax.clear_caches()
except Exception:
    pass


def get_nc():
    if not _NC_CACHE:
        _NC_CACHE.append(build_nc())
    return _NC_CACHE[0]


def run(in_maps, **kwargs):
    return run_bass_kernel_spmd(get_nc(), in_maps, core_ids=list(range(B)), **kwargs)


def kernel(x, Wqkv, Wo, bo):
    in_maps = host_inputs(x, Wqkv, Wo, bo)
    r = run(in_maps)
    y = np.stack([r.results[b]["y"].reshape(C, H, W) for b in range(B)])
    return y.astype(np.float32)


if __name__ == "__main__":
    nc = build_nc()
    print("built ok:", len(nc.inst_map), "instructions")



# revision 10
# speedup vs baseline: 1.4564x; 1.4564x over previous
"""Trainium2 Bass kernel for nn_AttentionBlock (B=8, C=512, H=W=32, 8 heads).

Sharding: data-parallel over batch — core b computes batch image b end-to-end
(weights replicated to all 8 cores).

Design notes (cost model: matmul time = moving-operand columns only; moving
dtype sets the rate, bf16 = 1 col/cycle at any N):
  P1a: q,k = Wqk^T.T @ x -> (1024, S), channel order arranged on host so each
       128-row tile is one head-PAIR of q or k.  All operands bf16.
  P1b: vT = x.T @ Wv^T -> (S, 512) so attention needs no transposes.
  sc : scoresT[key, q] per (j-tile, head): 2 matmuls of N=512 into a PSUM
       ping-pong slot; ACT exp -> et (bf16, SBUF) at scale 1/8.
  AV : FLIPPED vs the usual orientation — out[q-tile, d] = et-chunk^T @ v
       streams only N=64 v-columns per (head, j, q-chunk): 8x less moving
       data than streaming queries.  Denominators via extra N=1 matmuls
       (rhs = ones column) into a dedicated PSUM bank.
       AV for pair p runs one pair-loop LATE (during pair p+1's score/exp
       rounds) so the PE always has exp-independent work while ACT (the
       64x 1038ns exp stream) paces the attention phase.
  norm: DVE reciprocal of the 16 denominator columns, then one stride-0
       broadcast tensor_tensor per head: evict+normalize PSUM->SBUF bf16.
  T  : PE transposes (bf16 identity moving => 1 cycle/row) turn res'[q, c]
       into res[c, q] for the output projection.
  P4 : y = Wo^T.T @ resT + bo + x, STT on DVE, y DMA per m-tile.

PSUM banks: 0-3 scores ping-pong (2 slots x 4KB), 4-5 AV accumulators
(2 heads x 8 q-tiles x 64), 6 P1/transpose scratch, 7 denominators.
Interleaved PSUM accumulation groups share banks via the 2KB zero-region
rule: only the first matmul touching a bank-epoch uses start=True; other
groups' first writes auto-zero through the pending-zero flag.
"""

import os
import sys

for _p in ("/opt/trn_rl_repo", "/root/.axon_site/_ro/trn_rl_repo"):
    if os.path.isdir(_p) and _p not in sys.path:
        sys.path.insert(0, _p)

from contextlib import ExitStack

import ml_dtypes
import numpy as np

import concourse.bass as bass
import concourse.tile as tile
from concourse import mybir
from concourse.bass_utils import run_bass_kernel_spmd

B, C, H, W = 8, 512, 32, 32
NH, D = 8, 64
S = H * W            # 1024 sequence positions
P = 128              # partitions
KT = C // P          # 4 contraction tiles over channels
NT = S // P          # 8 key/query tiles
NPAIR = NH // 2      # 4 head pairs
NWCOL = 4 * C + P    # wallb cols: 2C qk | C v | C wo | 128 identity
F32 = mybir.dt.float32
BF16 = mybir.dt.bfloat16
AF = mybir.ActivationFunctionType
ALU = mybir.AluOpType

EXP_BUFS = int(os.environ.get("K_EXP_BUFS", "24"))
P1A_J = tuple(int(c) for c in os.environ.get("K_P1A_J", "1346"))


def _install_drain_split():
    """walrus's CTRL_NO (drain) codegen accepts only a single semaphore wait,
    but Tile's kernel-tail drain aggregates one wait per live proc.  Split
    them across several serial drains (semantically identical: all complete
    before the closing all-engine barrier)."""
    if getattr(tile.TileContext, "_drain_split_installed", False):
        return
    from concourse.vector_clock import ScopedClock

    orig = tile.TileContext._drain_and_barrier

    def patched(self, tick_clock, wait_clock):
        nc = self.nc
        drain_inst = nc.sync.drain()
        wait_clock.add_sem_waits(
            drain_inst.ins, ScopedClock({None: tick_clock.global_clock})
        )
        si = drain_inst.ins.sync_info
        if si is not None and si.on_wait and len(si.on_wait) > 1:
            waits = list(si.on_wait)
            drain_inst.ins.sync_info = mybir.SyncInfo(
                on_wait=[waits[0]], on_update=list(si.on_update or [])
            )
            for w in waits[1:]:
                d2 = nc.sync.drain()
                d2.ins.sync_info = mybir.SyncInfo(on_wait=[w], on_update=[])

        nc.all_engine_barrier()
        assert self.sems is not None
        popped = nc._tile_sem_poison_stack.pop()
        assert popped is self._sem_poison
        nc.clear_and_free_semaphores(list(self.sems.allocated().values()))
        nc.all_engine_barrier()

    tile.TileContext._drain_and_barrier = patched
    tile.TileContext._drain_split_installed = True
    tile.TileContext._drain_and_barrier_orig = orig


def trace_kernel(ctx, tc, nc, xb, wallb, bof, y):
    cst = ctx.enter_context(tc.tile_pool(name="cst", bufs=1))
    qkp = ctx.enter_context(tc.tile_pool(name="qkp", bufs=4))
    expp = ctx.enter_context(tc.tile_pool(name="expp", bufs=EXP_BUFS))
    rdp = ctx.enter_context(tc.tile_pool(name="rdp", bufs=2))
    rqp = ctx.enter_context(tc.tile_pool(name="rqp", bufs=2))
    yp = ctx.enter_context(tc.tile_pool(name="yp", bufs=1))
    # PSUM pools, allocation order = bank order (8 banks total):
    scp = ctx.enter_context(tc.tile_pool(name="scp", bufs=2, space="PSUM"))
    oap = ctx.enter_context(tc.tile_pool(name="oap", bufs=1, space="PSUM"))
    p1p = ctx.enter_context(tc.tile_pool(name="p1p", bufs=1, space="PSUM"))
    dnp = ctx.enter_context(tc.tile_pool(name="dnp", bufs=1, space="PSUM"))

    xt = cst.tile([P, KT, S], BF16)
    wall = cst.tile([P, KT, NWCOL], BF16)
    wqkt = wall[:, :, 0:2 * C]
    wvt = wall[:, :, 2 * C:3 * C]
    wot = wall[:, :, 3 * C:4 * C]
    ident = wall[:, 0, 4 * C:4 * C + P]          # [128, 128] bf16 identity

    bo_sb = cst.tile([P, KT], F32)
    onesc = cst.tile([P, 1], BF16)
    vta = cst.tile([P, NT, C], BF16)             # v^T tiles, head-major cols
    resT = cst.tile([P, KT, S], BF16)            # res[c, s], k-tile = pair
    scr = cst.tile([1, 256], F32)
    identb = cst.tile([P, P], BF16)
    ybig = yp.tile([P, KT, S], F32)

    # ---- input DMA, chunked so the first p1a epoch starts early ----
    xr = xb.rearrange("(k p) s -> p k s", p=P)
    wr = wallb.rearrange("(k p) c -> p k c", p=P)
    nc.sync.dma_start(out=xt[:, :, 0:512], in_=xr[:, :, 0:512])
    nc.gpsimd.dma_start(out=wall[:, :, 0:256], in_=wr[:, :, 0:256])
    nc.sync.dma_start(out=xt[:, :, 512:S], in_=xr[:, :, 512:S])
    nc.gpsimd.dma_start(out=wall[:, :, 2 * C:3 * C], in_=wr[:, :, 2 * C:3 * C])
    nc.gpsimd.dma_start(out=wall[:, :, 256:2 * C], in_=wr[:, :, 256:2 * C])
    nc.gpsimd.dma_start(out=wall[:, :, 3 * C:NWCOL], in_=wr[:, :, 3 * C:NWCOL])
    nc.gpsimd.dma_start(out=bo_sb.unsqueeze(2),
                        in_=bof.rearrange("(k p) o -> p k o", p=P))

    nc.vector.memset(onesc[:, :], 1.0)

    scr_i = [0]
    sync_done = set()

    def dve_sync(*aps):
        # DVE wait-carrier: absorb one cross-engine wait per tiny copy.
        for ap in aps:
            n = min(ap.free_size(), 8)
            o = (scr_i[0] % 30) * 8
            scr_i[0] += 1
            nc.vector.tensor_copy(scr[0:1, o:o + n], ap)

    def pe_mm(corner, dep):
        # PE wait-carrier: a 1x2 matmul reading `dep` absorbs one cross-
        # engine wait; PE program order subsumes the tick for later matmuls.
        # `corner` must be a PSUM slice fully rewritten by the next
        # start=True group in its bank.
        nc.tensor.matmul(
            corner, dep[0:1, 0:1], dep[0:1, 0:2],
            start=True, stop=True, skip_group_check=True,
        )

    # ---------------- P1a: q,k projection epochs ----------------
    qk_tiles = [None] * NPAIR

    def p1a_epoch(m, n, first=False):
        """One (m-tile, n-half) epoch: 4 matmuls into the b6 scratch bank,
        then DVE-evict to the pair's qk tile (bf16)."""
        pair, isk = divmod(m, 2)
        if isk == 0 and n == 0:
            qk_tiles[pair] = qkp.tile([P, 2 * S], BF16, tag="qk",
                                      name=f"qk{pair}")
        acc = p1p.tile([P, 512], F32, tag="p1", name=f"p1a{m}_{n}")
        if first:
            # first matmul of the kernel: absorb the two input-DMA waits
            pe_mm(acc[0:1, 0:2], xt[:, 0, 0:2])
            pe_mm(acc[0:1, 0:2], wall[:, 0, 0:2])
        for k in range(KT):
            nc.tensor.matmul(
                acc[:, :],
                wqkt[:, k, m * P:(m + 1) * P],
                xt[:, k, n * 512:(n + 1) * 512],
                start=(k == 0), stop=(k == KT - 1),
            )
        with nc.allow_low_precision(reason="bf16 qk tiles"):
            nc.vector.tensor_copy(
                qk_tiles[pair][:, isk * S + n * 512: isk * S + (n + 1) * 512],
                acc[:, :],
            )

    # ---------------- P1b: v projection (inside pair 0 loop) ----------------
    def p1b_epoch(j):
        acc = p1p.tile([P, 512], F32, tag="p1", name=f"p1b{j}")
        for k in range(KT):
            nc.tensor.matmul(
                acc[:, :],
                xt[:, k, j * P:(j + 1) * P],
                wvt[:, k, :],
                start=(k == 0), stop=(k == KT - 1),
            )
        with nc.allow_low_precision(reason="bf16 v tiles"):
            nc.vector.tensor_copy(vta[:, j, :], acc[:, :])

    # ---------------- attention state ----------------
    pending_pe_syncs = []           # dep APs to absorb via sc-corner pe_mm
    ets_hist = []                   # every et tile in emission order
    pair_ets = [[None] * (2 * NT) for _ in range(NPAIR)]  # [pair][2*j+hh]
    oa_cur = [None]                 # AV accumulators of the lagging pair
    den_cur = [None]

    def scores_round(pair, j):
        """scores + exp for (pair, j), both heads. Returns nothing."""
        qk = qk_tiles[pair]
        for hh in range(2):
            sc = scp.tile([P, S], F32, tag="sc", name=f"sc{pair}_{j}_{hh}")
            idx = len(ets_hist)
            if idx >= 2:
                # WAR carrier: this slot was last read by exp #idx-2
                pe_mm(sc[0:1, 0:2], ets_hist[idx - 2])
            while pending_pe_syncs:
                pe_mm(sc[0:1, 0:2], pending_pe_syncs.pop())
            for n in range(2):
                nc.tensor.matmul(
                    sc[:, n * 512:(n + 1) * 512],
                    qk[64 * hh:64 * (hh + 1), S + j * P: S + (j + 1) * P],
                    qk[64 * hh:64 * (hh + 1), n * 512:(n + 1) * 512],
                    start=True, stop=True,
                )
            et = expp.tile([P, S], BF16, tag="et", name=f"et{pair}_{j}_{hh}")
            nc.scalar.activation(et[:, :], sc[:, :], AF.Exp, scale=1.0 / np.sqrt(D))
            ets_hist.append(et)
            pair_ets[pair][2 * j + hh] = et

    def av_open(pair):
        oa_cur[0] = oap.tile([P, 2 * 512], F32, tag="oa", name=f"oa{pair}")
        den_cur[0] = dnp.tile([P, 16], F32, tag="den", name=f"den{pair}")

    def av_round(pair, j):
        """AV + denominator matmuls for (pair, j) using saved et tiles."""
        oa, den = oa_cur[0], den_cur[0]
        for hh in range(2):
            et = pair_ets[pair][2 * j + hh]
            v = vta[:, j, (2 * pair + hh) * D:(2 * pair + hh + 1) * D]
            for t in range(NT):
                nc.tensor.matmul(
                    oa[:, hh * 512 + t * D: hh * 512 + (t + 1) * D],
                    et[:, t * P:(t + 1) * P], v,
                    start=(j == 0 and t == 0), stop=(j == NT - 1),
                    skip_group_check=True,
                )
                nc.tensor.matmul(
                    den[:, hh * NT + t: hh * NT + t + 1],
                    et[:, t * P:(t + 1) * P], onesc[:, :],
                    start=(j == 0 and t == 0 and hh == 0),
                    stop=(j == NT - 1),
                    skip_group_check=True,
                )

    def norm_pair(pair):
        """reciprocal + evict/normalize oa -> resq (bf16), then transposes
        into b6 and evict to resT[:, pair, :]."""
        oa, den = oa_cur[0], den_cur[0]
        rd = rdp.tile([P, 16], F32, tag="rd", name=f"rd{pair}")
        nc.vector.reciprocal(rd[:, :], den[:, :])
        resq = rqp.tile([P, NT, P], BF16, tag="rq", name=f"resq{pair}")
        with nc.allow_low_precision(reason="bf16 res tiles"):
            for hh in range(2):
                nc.vector.tensor_tensor(
                    resq[:, :, hh * D:(hh + 1) * D],
                    oa[:, hh * 512:(hh + 1) * 512].rearrange(
                        "p (t d) -> p t d", t=NT),
                    rd[:, hh * NT:(hh + 1) * NT].unsqueeze(2).broadcast_to(
                        [P, NT, D]),
                    op=ALU.mult,
                )
        return resq

    def transpose_pair(pair, resq):
        tp = p1p.tile([P, NT * P], BF16, tag="p1", name=f"tp{pair}")
        for t in range(NT):
            nc.tensor.transpose(
                tp[:, t * P:(t + 1) * P], resq[:, t, :], identb[:, :])
        nc.vector.tensor_copy(resT[:, pair, :], tp[:, :])

    # ================= schedule =================
    # pre-loop: pair 0 q,k.  The dummy scp tile is a safe corner target for
    # pre-attention PE wait-carriers (its slot is fully rewritten by the
    # first scores matmuls' start=True groups).
    dummy = scp.tile([P, S], F32, tag="sc", name="dummy")
    dve_sync(xt[0:1, 0, 0:8])          # DVE absorbs xt chunk 1 DMA wait
    dve_sync(xt[0:1, 0, 512:520])      # DVE absorbs xt chunk 2 DMA wait
    p1a_epoch(0, 0, first=True)
    p1a_epoch(1, 0)
    pe_mm(dummy[0:1, 0:2], xt[:, 0, 512:514])   # PE absorbs xt chunk 2 wait
    p1a_epoch(0, 1)
    p1a_epoch(1, 1)
    pending_pe_syncs.append(wvt[:, 0, 0:2])     # for p1b (wvt DMA chunk)

    resq_prev = None          # (pair_idx, resq) awaiting transpose
    norm_tick = None          # ap proving prev norm read oa (for WAR carrier)

    def oa_war_carrier():
        # oa/den WAR: prev pair's norm (DVE) must be done before this
        # pair's AV epoch opens the banks; absorb that single DVE wait.
        # The corner write is re-zeroed by AV j0 t0's start=True.
        nc.tensor.matmul(
            oa_cur[0][0:1, 0:2], norm_tick[0:1, 0, 0:1],
            norm_tick[0:1, 0, 0:2],
            start=True, stop=True, skip_group_check=True,
        )

    for pair in range(NPAIR):
        av_pair = pair - 1    # AV lags one pair behind
        if pair == 2:
            # copy the identity into a DVE-produced tile: the transposes then
            # carry a single merged DVE wait (ident-copy + resq + b6 WAR)
            nc.vector.tensor_copy(identb[:, :], ident)
        if av_pair >= 0:
            av_open(av_pair)
        for j in range(NT):
            scores_round(pair, j)
            if pair == 0:
                p1b_epoch(j)
            if pair < NPAIR - 1 and j in P1A_J:
                # q,k projection for the next pair
                i = P1A_J.index(j)
                p1a_epoch(2 * (pair + 1) + (i // 2), i % 2)
            if av_pair >= 0:
                if j == 0 and norm_tick is not None:
                    oa_war_carrier()
                av_round(av_pair, j)
            if resq_prev is not None and j == 1:
                tr_pair, tr_resq = resq_prev
                transpose_pair(tr_pair, tr_resq)
                resq_prev = None
        if av_pair >= 0:
            resq = norm_pair(av_pair)
            norm_tick = resq
            resq_prev = (av_pair, resq)

    # tail: AV + norm + transpose for the last pair
    av_open(NPAIR - 1)
    oa_war_carrier()
    for j in range(NT):
        av_round(NPAIR - 1, j)
    if resq_prev is not None:
        tr_pair, tr_resq = resq_prev
        transpose_pair(tr_pair, tr_resq)
        resq_prev = None
    resq = norm_pair(NPAIR - 1)
    transpose_pair(NPAIR - 1, resq)

    # ---------------- P4: output projection + bias + residual ----------------
    dve_sync(bo_sb[0:1, 0:1])
    yr = y.rearrange("(k p) s -> p k s", p=P)
    for m in range(KT):
        acc = scp.tile([P, S], F32, tag="sc", name=f"p4acc{m}")
        if m == 0:
            # absorb resT evict (DVE) into a carrier; scp WAR (exp) too
            pe_mm(acc[0:1, 0:2], ets_hist[-1])
            pe_mm(acc[0:1, 0:2], resT[:, NPAIR - 1, :])
        for n in range(2):
            for k in range(KT):
                nc.tensor.matmul(
                    acc[:, n * 512:(n + 1) * 512],
                    wot[:, k, m * P:(m + 1) * P],
                    resT[:, k, n * 512:(n + 1) * 512],
                    start=(k == 0), stop=(k == KT - 1),
                )
        nc.vector.scalar_tensor_tensor(
            ybig[:, m, :], acc[:, :], bo_sb[:, m:m + 1],
            xt[:, m, :], op0=ALU.add, op1=ALU.add,
        )
        nc.sync.dma_start(out=yr[:, m:m + 1, :], in_=ybig[:, m:m + 1, :])


ENGINE_SEM_PREFIX = {
    "PE": "PE_",
    "Activation": "Activation_",
    "DVE": "DVE_",
    "Pool": "Pool_",
    "SP": "SP_",
}


def _strip_self_waits(nc):
    """Drop same-engine semaphore self-waits from multi-wait instructions.

    Engines execute and complete their own instructions in program order,
    so a wait on the engine's own completion semaphore is redundant whenever
    the instruction carries another wait — and walrus's PE/ACT instruction
    structs only encode a single wait.
    """
    n = 0
    for inst in nc.inst_map.values():
        si = getattr(inst, "sync_info", None)
        if si is None or not si.on_wait or len(si.on_wait) <= 1:
            continue
        eng = str(getattr(inst, "engine", "")).split(".")[-1]
        pref = ENGINE_SEM_PREFIX.get(eng)
        if pref is None:
            continue
        keep = [w for w in si.on_wait if not w.ant_name.startswith(pref)]
        if len(keep) != len(si.on_wait) and keep:
            inst.sync_info = mybir.SyncInfo(
                on_wait=keep, on_update=list(si.on_update or [])
            )
            n += 1
    return n


def build_nc():
    _install_drain_split()
    nc = bass.Bass(trn_type="TRN2", debug=False, num_devices=8)
    x_d = nc.dram_tensor("xb", [C, S], BF16, kind="ExternalInput")
    w_d = nc.dram_tensor("wallb", [C, NWCOL], BF16, kind="ExternalInput")
    b_d = nc.dram_tensor("bof", [C, 1], F32, kind="ExternalInput")
    y_d = nc.dram_tensor("y", [C, S], F32, kind="ExternalOutput")
    with tile.TileContext(nc) as tc, ExitStack() as ctx:
        trace_kernel(ctx, tc, nc, x_d.ap(), w_d.ap(), b_d.ap(), y_d.ap())
    _strip_self_waits(nc)
    if not nc.is_finalized():
        nc.finalize()
    return nc


def host_inputs(x, Wqkv, Wo, bo):
    """Host-side reshard: per-core input dicts (weights replicated)."""
    x = np.asarray(x, dtype=np.float32)
    Wqkv = np.asarray(Wqkv, dtype=np.float32)
    Wo = np.asarray(Wo, dtype=np.float32)
    bo = np.asarray(bo, dtype=np.float32)

    # Wqkv rows per head h: [h*3D, h*3D+D) = q, [+D, +2D) = k, [+2D, +3D) = v.
    # q,k channel order: per pair -> [q(2p)|q(2p+1)], [k(2p)|k(2p+1)] tiles.
    order = []
    for p in range(NPAIR):
        for h in (2 * p, 2 * p + 1):
            order.extend(range(h * 3 * D, h * 3 * D + D))          # q rows
        for h in (2 * p, 2 * p + 1):
            order.extend(range(h * 3 * D + D, h * 3 * D + 2 * D))  # k rows
    wqkt = Wqkv[order].T                                            # (C, 2C)
    v_order = [h * 3 * D + 2 * D + d for h in range(NH) for d in range(D)]
    wvt = Wqkv[v_order].T                                           # (C, C)
    wot = Wo.T                                                      # (C, C)
    ident = np.zeros((C, P), dtype=np.float32)
    ident[0:P, 0:P] = np.eye(P)
    wallb = np.ascontiguousarray(
        np.concatenate([wqkt, wvt, wot, ident], axis=1)
    ).astype(ml_dtypes.bfloat16)                                    # (C, 4C+128)
    bof = np.ascontiguousarray(bo[:, None])                         # (C, 1)

    xb = x.reshape(B, C, S).astype(ml_dtypes.bfloat16)
    return [
        dict(xb=np.ascontiguousarray(xb[b]), wallb=wallb, bof=bof)
        for b in range(B)
    ]


_NC_CACHE = []

try:
    # bass_exec HLO does not embed the BIR; bust jax's executable cache so a
    # rebuilt kernel is actually recompiled instead of hitting a stale NEFF.
    import jax as _jax

    _jax.clear_caches()
except Exception:
    pass


def get_nc():
    if not _NC_CACHE:
        _NC_CACHE.append(build_nc())
    return _NC_CACHE[0]


def run(in_maps, **kwargs):
    return run_bass_kernel_spmd(get_nc(), in_maps, core_ids=list(range(B)), **kwargs)


def kernel(x, Wqkv, Wo, bo):
    in_maps = host_inputs(x, Wqkv, Wo, bo)
    r = run(in_maps)
    yv = np.stack([r.results[b]["y"].reshape(C, H, W) for b in range(B)])
    return yv.astype(np.float32)


if __name__ == "__main__":
    nc = build_nc()
    print("built ok:", len(nc.inst_map), "instructions")


# revision 18
# speedup vs baseline: 1.4895x; 1.0227x over previous
"""Trainium2 Bass kernel for nn_AttentionBlock (B=8, C=512, H=W=32, 8 heads).

Sharding: data-parallel over batch — core b computes batch image b end-to-end
(weights replicated to all 8 cores).

Design notes (cost model: matmul time = moving-operand columns only; moving
dtype sets the rate, bf16 = 1 col/cycle at any N):
  P1a: q,k = Wqk^T.T @ x -> (1024, S), channel order arranged on host so each
       128-row tile is one head-PAIR of q or k.  All operands bf16.
  P1b: vT = x.T @ Wv^T -> (S, 512) so attention needs no transposes.
  sc : scoresT[key, q] per (j-tile, head): 2 matmuls of N=512 into a PSUM
       ping-pong slot; ACT exp -> et (bf16, SBUF) at scale 1/8.
  AV : FLIPPED vs the usual orientation — out[q-tile, d] = et-chunk^T @ v
       streams only N=64 v-columns per (head, j, q-chunk): 8x less moving
       data than streaming queries.  Denominators via extra N=1 matmuls
       (rhs = ones column) into a dedicated PSUM bank.
       AV for pair p runs one pair-loop LATE (during pair p+1's score/exp
       rounds) so the PE always has exp-independent work while ACT (the
       64x 1038ns exp stream) paces the attention phase.
  norm: DVE reciprocal of the 16 denominator columns, then one stride-0
       broadcast tensor_tensor per head: evict+normalize PSUM->SBUF bf16.
  T  : PE transposes (bf16 identity moving => 1 cycle/row) turn res'[q, c]
       into res[c, q] for the output projection.
  P4 : y = Wo^T.T @ resT + bo + x, STT on DVE, y DMA per m-tile.

PSUM banks: 0-3 scores ping-pong (2 slots x 4KB), 4-5 AV accumulators
(2 heads x 8 q-tiles x 64), 6 P1/transpose scratch, 7 denominators.
Interleaved PSUM accumulation groups share banks via the 2KB zero-region
rule: only the first matmul touching a bank-epoch uses start=True; other
groups' first writes auto-zero through the pending-zero flag.
"""

import os
import sys

for _p in ("/opt/trn_rl_repo", "/root/.axon_site/_ro/trn_rl_repo"):
    if os.path.isdir(_p) and _p not in sys.path:
        sys.path.insert(0, _p)

from contextlib import ExitStack

import ml_dtypes
import numpy as np

import concourse.bass as bass
import concourse.tile as tile
from concourse import mybir
from concourse.bass_utils import run_bass_kernel_spmd

B, C, H, W = 8, 512, 32, 32
NH, D = 8, 64
S = H * W            # 1024 sequence positions
P = 128              # partitions
KT = C // P          # 4 contraction tiles over channels
NT = S // P          # 8 key/query tiles
NPAIR = NH // 2      # 4 head pairs
NWCOL = 4 * C + P    # wallb cols: 2C qk | C v | C wo | 128 identity
F32 = mybir.dt.float32
BF16 = mybir.dt.bfloat16
AF = mybir.ActivationFunctionType
ALU = mybir.AluOpType

EXP_BUFS = int(os.environ.get("K_EXP_BUFS", "24"))
WARM_BIG = int(os.environ.get("K_WARM_BIG", "8"))
WARM_SMALL = int(os.environ.get("K_WARM_SMALL", "4"))


def _install_drain_split():
    """walrus's CTRL_NO (drain) codegen accepts only a single semaphore wait,
    but Tile's kernel-tail drain aggregates one wait per live proc.  Split
    them across several serial drains (semantically identical: all complete
    before the closing all-engine barrier)."""
    if getattr(tile.TileContext, "_drain_split_installed", False):
        return
    from concourse.vector_clock import ScopedClock

    orig = tile.TileContext._drain_and_barrier

    def patched(self, tick_clock, wait_clock):
        nc = self.nc
        drain_inst = nc.sync.drain()
        wait_clock.add_sem_waits(
            drain_inst.ins, ScopedClock({None: tick_clock.global_clock})
        )
        si = drain_inst.ins.sync_info
        if si is not None and si.on_wait and len(si.on_wait) > 1:
            waits = list(si.on_wait)
            drain_inst.ins.sync_info = mybir.SyncInfo(
                on_wait=[waits[0]], on_update=list(si.on_update or [])
            )
            for w in waits[1:]:
                d2 = nc.sync.drain()
                d2.ins.sync_info = mybir.SyncInfo(on_wait=[w], on_update=[])

        nc.all_engine_barrier()
        assert self.sems is not None
        popped = nc._tile_sem_poison_stack.pop()
        assert popped is self._sem_poison
        nc.clear_and_free_semaphores(list(self.sems.allocated().values()))
        nc.all_engine_barrier()

    tile.TileContext._drain_and_barrier = patched
    tile.TileContext._drain_split_installed = True
    tile.TileContext._drain_and_barrier_orig = orig


def trace_kernel(ctx, tc, nc, xb, wallb, bof, y):
    cst = ctx.enter_context(tc.tile_pool(name="cst", bufs=1))
    qkp = ctx.enter_context(tc.tile_pool(name="qkp", bufs=4))
    expp = ctx.enter_context(tc.tile_pool(name="expp", bufs=EXP_BUFS))
    rdp = ctx.enter_context(tc.tile_pool(name="rdp", bufs=2))
    rqp = ctx.enter_context(tc.tile_pool(name="rqp", bufs=2))
    yp = ctx.enter_context(tc.tile_pool(name="yp", bufs=1))
    # PSUM pools, allocation order = bank order (8 banks total):
    scp = ctx.enter_context(tc.tile_pool(name="scp", bufs=2, space="PSUM"))
    oap = ctx.enter_context(tc.tile_pool(name="oap", bufs=1, space="PSUM"))
    p1p = ctx.enter_context(tc.tile_pool(name="p1p", bufs=1, space="PSUM"))
    dnp = ctx.enter_context(tc.tile_pool(name="dnp", bufs=1, space="PSUM"))

    xt = cst.tile([P, KT, S], BF16)
    wall = cst.tile([P, KT, NWCOL], BF16)
    wqkt = wall[:, :, 0:2 * C]
    wvt = wall[:, :, 2 * C:3 * C]
    wot = wall[:, :, 3 * C:4 * C]
    ident = wall[:, 0, 4 * C:4 * C + P]          # [128, 128] bf16 identity

    bo_sb = cst.tile([P, KT], F32)
    onesc = cst.tile([P, 1], BF16)
    vta = cst.tile([P, NT, C], BF16)             # v^T tiles, head-major cols
    resT = cst.tile([P, KT, S], BF16)            # res[c, s], k-tile = pair
    scr = cst.tile([1, 256], F32)
    identb = cst.tile([P, P], BF16)
    warm = cst.tile([P, 640], BF16)
    yax = cst.tile([P, KT, S], F32)              # yA partial (k=0,1) + x
    ybig = yp.tile([P, KT, S], BF16)

    # ---- input DMA, chunked so the first p1a epoch starts early ----
    xr = xb.rearrange("(k p) s -> p k s", p=P)
    wr = wallb.rearrange("(k p) c -> p k c", p=P)
    nc.gpsimd.dma_start(out=wall[:, :, 0:256], in_=wr[:, :, 0:256])
    nc.sync.dma_start(out=xt[:, :, 0:512], in_=xr[:, :, 0:512])
    nc.sync.dma_start(out=xt[:, :, 512:S], in_=xr[:, :, 512:S])
    nc.gpsimd.dma_start(out=wall[:, :, 2 * C:3 * C], in_=wr[:, :, 2 * C:3 * C])
    nc.gpsimd.dma_start(out=wall[:, :, 256:2 * C], in_=wr[:, :, 256:2 * C])
    nc.gpsimd.dma_start(out=wall[:, :, 3 * C:NWCOL], in_=wr[:, :, 3 * C:NWCOL])
    nc.gpsimd.dma_start(out=bo_sb.unsqueeze(2),
                        in_=bof.rearrange("(k p) o -> p k o", p=P))

    nc.vector.memset(onesc[:, :], 1.0)

    scr_i = [0]
    sync_done = set()

    def dve_sync(*aps):
        # DVE wait-carrier: absorb one cross-engine wait per tiny copy.
        # Callers pass 2-D APs (partition x free).
        for ap in aps:
            n = min(ap.free_size(), 16)
            o = (scr_i[0] % 15) * 16
            scr_i[0] += 1
            nc.vector.tensor_copy(scr[0:1, o:o + n], ap[0:1, 0:n])

    def pe_mm(corner, dep):
        # PE wait-carrier: a 1x2 matmul reading `dep` absorbs one cross-
        # engine wait; PE program order subsumes the tick for later matmuls.
        # `corner` must be a PSUM slice fully rewritten by the next
        # start=True group in its bank.
        nc.tensor.matmul(
            corner, dep[0:1, 0:1], dep[0:1, 0:2],
            start=True, stop=True, skip_group_check=True,
        )

    # ---------------- P1a: q,k projection epochs ----------------
    qk_tiles = [None] * NPAIR

    def p1a_epoch(m, n, first=False):
        """One (m-tile, n-half) epoch: 4 matmuls into the b6 scratch bank,
        then DVE-evict to the pair's qk tile (bf16)."""
        pair, isk = divmod(m, 2)
        if isk == 0 and n == 0:
            qk_tiles[pair] = qkp.tile([P, 2 * S], BF16, tag="qk",
                                      name=f"qk{pair}")
        acc = p1p.tile([P, 512], F32, tag="p1", name=f"p1a{m}_{n}")
        if first:
            # first matmul of the kernel: absorb the two input-DMA waits
            pe_mm(acc[0:1, 0:2], xt[:, 0, 0:2])
            pe_mm(acc[0:1, 0:2], wall[:, 0, 0:2])
        for k in range(KT):
            nc.tensor.matmul(
                acc[:, :],
                wqkt[:, k, m * P:(m + 1) * P],
                xt[:, k, n * 512:(n + 1) * 512],
                start=(k == 0), stop=(k == KT - 1),
            )
        with nc.allow_low_precision(reason="bf16 qk tiles"):
            nc.vector.tensor_copy(
                qk_tiles[pair][:, isk * S + n * 512: isk * S + (n + 1) * 512],
                acc[:, :],
            )

    def p1a_quarter(m, q):
        """Quarter (N=256) p1a epoch: smaller b6 residency for the packed
        pair-0 loop."""
        pair, isk = divmod(m, 2)
        if isk == 0 and q == 0:
            qk_tiles[pair] = qkp.tile([P, 2 * S], BF16, tag="qk",
                                      name=f"qk{pair}")
        acc = p1p.tile([P, 256], F32, tag="p1", name=f"p1aq{m}_{q}")
        for k in range(KT):
            nc.tensor.matmul(
                acc[:, :],
                wqkt[:, k, m * P:(m + 1) * P],
                xt[:, k, q * 256:(q + 1) * 256],
                start=(k == 0), stop=(k == KT - 1),
            )
        with nc.allow_low_precision(reason="bf16 qk tiles"):
            nc.vector.tensor_copy(
                qk_tiles[pair][:, isk * S + q * 256: isk * S + (q + 1) * 256],
                acc[:, :],
            )

    # ---------------- P1b: v projection (inside pair 0 loop) ----------------
    def p1b_epoch(j):
        acc = p1p.tile([P, 512], F32, tag="p1", name=f"p1b{j}")
        for k in range(KT):
            nc.tensor.matmul(
                acc[:, :],
                xt[:, k, j * P:(j + 1) * P],
                wvt[:, k, :],
                start=(k == 0), stop=(k == KT - 1),
            )
        with nc.allow_low_precision(reason="bf16 v tiles"):
            nc.vector.tensor_copy(vta[:, j, :], acc[:, :])

    # ---------------- attention state ----------------
    pending_pe_syncs = []           # dep APs to absorb via sc-corner pe_mm
    ets_hist = []                   # every et tile in emission order
    pair_ets = [[None] * (2 * NT) for _ in range(NPAIR)]  # [pair][2*j+hh]
    oa_cur = [None]                 # AV accumulators of the lagging pair
    den_cur = [None]

    def scores_round(pair, j):
        """scores + exp for (pair, j), both heads. Returns nothing."""
        qk = qk_tiles[pair]
        for hh in range(2):
            sc = scp.tile([P, S], F32, tag="sc", name=f"sc{pair}_{j}_{hh}")
            idx = len(ets_hist)
            if idx >= 2:
                # WAR carrier: this slot was last read by exp #idx-2
                pe_mm(sc[0:1, 0:2], ets_hist[idx - 2])
            while pending_pe_syncs:
                pe_mm(sc[0:1, 0:2], pending_pe_syncs.pop())
            for n in range(2):
                nc.tensor.matmul(
                    sc[:, n * 512:(n + 1) * 512],
                    qk[64 * hh:64 * (hh + 1), S + j * P: S + (j + 1) * P],
                    qk[64 * hh:64 * (hh + 1), n * 512:(n + 1) * 512],
                    start=True, stop=True,
                )
            et = expp.tile([P, S], BF16, tag="et", name=f"et{pair}_{j}_{hh}")
            nc.scalar.activation(et[:, :], sc[:, :], AF.Exp, scale=1.0 / np.sqrt(D))
            ets_hist.append(et)
            pair_ets[pair][2 * j + hh] = et

    def av_open(pair):
        oa_cur[0] = oap.tile([P, 2 * 512], F32, tag="oa", name=f"oa{pair}")
        den_cur[0] = dnp.tile([P, 16], F32, tag="den", name=f"den{pair}")

    def av_round(pair, j):
        """AV + denominator matmuls for (pair, j) using saved et tiles."""
        oa, den = oa_cur[0], den_cur[0]
        for hh in range(2):
            et = pair_ets[pair][2 * j + hh]
            v = vta[:, j, (2 * pair + hh) * D:(2 * pair + hh + 1) * D]
            for t in range(NT):
                nc.tensor.matmul(
                    oa[:, hh * 512 + t * D: hh * 512 + (t + 1) * D],
                    et[:, t * P:(t + 1) * P], v,
                    start=(j == 0 and t == 0), stop=(j == NT - 1),
                    skip_group_check=True,
                )
                nc.tensor.matmul(
                    den[:, hh * NT + t: hh * NT + t + 1],
                    et[:, t * P:(t + 1) * P], onesc[:, :],
                    start=(j == 0 and t == 0 and hh == 0),
                    stop=(j == NT - 1),
                    skip_group_check=True,
                )

    def ya_group(m, n):
        """P4 partial for k=0,1: 2 matmuls into b6, then STT-evict
        yax[:, m, n-half] = acc + x (residual folded here; bias at yB)."""
        acc = p1p.tile([P, 512], F32, tag="p1", name=f"ya{m}_{n}")
        for k in range(2):
            nc.tensor.matmul(
                acc[:, :],
                wot[:, k, m * P:(m + 1) * P],
                resT[:, k, n * 512:(n + 1) * 512],
                start=(k == 0), stop=(k == 1),
            )
        nc.vector.scalar_tensor_tensor(
            yax[:, m, n * 512:(n + 1) * 512], acc[:, :], 0.0,
            xt[:, m, n * 512:(n + 1) * 512], op0=ALU.add, op1=ALU.add,
        )

    def norm_pair(pair):
        """reciprocal + evict/normalize oa -> resq (bf16), then transposes
        into b6 and evict to resT[:, pair, :]."""
        oa, den = oa_cur[0], den_cur[0]
        # Sample-read DVE carriers absorbing the norm group's PE waits, one
        # per read region (den all, oa hh0, oa hh1): each carries a single
        # PE wait with the framework's schedule-correct value, after which
        # the recip and norm multiplies keep only single DVE waits.
        dve_sync(den[0:1, 0:16])
        for hh in range(2):
            dve_sync(oa[0:1, hh * 512:(hh + 1) * 512].rearrange(
                "p (t d) -> p t d", t=NT)[:, :, 0:1].rearrange(
                "p t d -> p (t d)"))
        rd = rdp.tile([P, 16], F32, tag="rd", name=f"rd{pair}")
        nc.vector.reciprocal(rd[:, :], den[:, :])
        resq = rqp.tile([P, NT, P], BF16, tag="rq", name=f"resq{pair}")
        with nc.allow_low_precision(reason="bf16 res tiles"):
            for hh in range(2):
                nc.vector.tensor_tensor(
                    resq[:, :, hh * D:(hh + 1) * D],
                    oa[:, hh * 512:(hh + 1) * 512].rearrange(
                        "p (t d) -> p t d", t=NT),
                    rd[:, hh * NT:(hh + 1) * NT].unsqueeze(2).broadcast_to(
                        [P, NT, D]),
                    op=ALU.mult,
                )
        return resq

    def transpose_pair(pair, resq):
        tp = p1p.tile([P, NT * P], BF16, tag="p1", name=f"tp{pair}")
        for t in range(NT):
            nc.tensor.transpose(
                tp[:, t * P:(t + 1) * P], resq[:, t, :], identb[:, :])
        nc.vector.tensor_copy(resT[:, pair, :], tp[:, :])

    # ================= schedule =================
    # The dummy scp tile is a safe corner target for pre-attention PE
    # wait-carriers AND the warm-up spin target (its slot is fully rewritten
    # by the first scores matmuls' start=True groups).
    dummy = scp.tile([P, S], F32, tag="sc", name="dummy")
    dve_sync(xt[0:1, 0, 0:8])          # DVE absorbs xt chunk 1 DMA wait
    dve_sync(xt[0:1, 0, 512:520])      # DVE absorbs xt chunk 2 DMA wait
    # warm-up: spin the PE on junk matmuls while the input DMA lands so the
    # p-state ramp (2x slower until ~3us of continuous busy) completes early
    nc.vector.memset(warm[:, :], 0.25)
    for i in range(WARM_BIG):
        nc.tensor.matmul(dummy[:, 0:512], warm[:, 0:128], warm[:, 128:640],
                         start=True, stop=True, skip_group_check=True)
    for i in range(WARM_SMALL):
        nc.tensor.matmul(dummy[:, 0:128], warm[:, 0:128], warm[:, 128:256],
                         start=True, stop=True, skip_group_check=True)
    # pre-loop: pair 0 q,k
    p1a_epoch(0, 0, first=True)
    p1a_epoch(1, 0)
    pe_mm(dummy[0:1, 0:2], xt[:, 0, 512:514])   # PE absorbs xt chunk 2 wait
    p1a_epoch(0, 1)
    p1a_epoch(1, 1)
    pending_pe_syncs.append(wvt[:, 0, 0:2])     # for p1b (wvt DMA chunk)
    dve_sync(bo_sb[0:1, 0:1])                   # bo DMA wait for yA STTs

    norm_tick = None          # ap proving prev norm read oa (for WAR carrier)

    def oa_war_carrier():
        # oa/den WAR: prev pair's norm (DVE) must be done before this
        # pair's AV epoch opens the banks; absorb that single DVE wait.
        # The corner write is re-zeroed by AV j0 t0's start=True.
        nc.tensor.matmul(
            oa_cur[0][0:1, 0:2], norm_tick[0:1, 0, 0:1],
            norm_tick[0:1, 0, 0:2],
            start=True, stop=True, skip_group_check=True,
        )

    # Per-loop round plan (pair p does scores/exp for itself; AV for p-1):
    #  r0-r3: AV(p-1) 2 j's per round         [p==0: p1b + p1a-quarters]
    #  r4   : norm(p-1)                       [p in 1,2: p1a half-epoch]
    #  r5   : transpose(p-1) + resT evict     [p==3: AV3 j0,j1]
    #  r6,r7: p1a half-epochs / yA / AV3
    resq_prev = None
    for pair in range(NPAIR):
        av_pair = pair - 1
        if pair == 1:
            # identity -> DVE-owned copy (transposes then carry one DVE wait)
            nc.vector.tensor_copy(identb[:, :], ident)
        if pair == 3:
            # wot DMA chunk wait for the yA matmuls
            pending_pe_syncs.append(wot[:, 0, 0:2])
        if av_pair >= 0:
            av_open(av_pair)
        for j in range(NT):
            scores_round(pair, j)
            if pair == 0:
                p1b_epoch(j)
                p1a_quarter(2 + j // 4, j % 4)          # pair 1 q,k
            if av_pair >= 0 and j < 4:
                if j == 0 and norm_tick is not None:
                    oa_war_carrier()
                av_round(av_pair, 2 * j)
                av_round(av_pair, 2 * j + 1)
            if pair in (1, 2) and j >= 4:
                p1a_epoch(2 * (pair + 1) + (j - 4) // 2, (j - 4) % 2)
            if pair == 2 and j >= 6:
                ya_group(j - 6, 0)
                ya_group(j - 6, 1)
            if pair == 3 and j < 2:
                ya_group(2 + j, 0)
                ya_group(2 + j, 1)
            if av_pair >= 0 and j == 4:
                resq = norm_pair(av_pair)
                norm_tick = resq
                resq_prev = (av_pair, resq)
            if av_pair >= 0 and j == 5:
                tr_pair, tr_resq = resq_prev
                transpose_pair(tr_pair, tr_resq)
                resq_prev = None
            if pair == 3 and j >= 5:
                if j == 5:
                    av_open(NPAIR - 1)
                    oa_war_carrier()
                    av_round(NPAIR - 1, 0)
                    av_round(NPAIR - 1, 1)
                elif j == 6:
                    av_round(NPAIR - 1, 2)
                    av_round(NPAIR - 1, 3)
                    av_round(NPAIR - 1, 4)
                else:
                    av_round(NPAIR - 1, 5)
                    av_round(NPAIR - 1, 6)

    # ---------------- tail ----------------
    av_round(NPAIR - 1, NT - 1)
    resq = norm_pair(NPAIR - 1)
    transpose_pair(NPAIR - 1, resq)

    # yB: remaining P4 contraction (k=2,3) + bias + yax, then y DMA per m
    yr = y.rearrange("(k p) s -> p k s", p=P)
    for m in range(KT):
        acc = scp.tile([P, S], F32, tag="sc", name=f"ybacc{m}")
        if m == 0:
            pe_mm(acc[0:1, 0:2], ets_hist[-1])
            pe_mm(acc[0:1, 0:2], resT[:, NPAIR - 1, :])
        for n in range(2):
            for k in (2, 3):
                nc.tensor.matmul(
                    acc[:, n * 512:(n + 1) * 512],
                    wot[:, k, m * P:(m + 1) * P],
                    resT[:, k, n * 512:(n + 1) * 512],
                    start=(k == 2), stop=(k == 3),
                )
        with nc.allow_low_precision(reason="bf16 y output"):
            nc.vector.scalar_tensor_tensor(
                ybig[:, m, :], acc[:, :], bo_sb[:, m:m + 1],
                yax[:, m, :], op0=ALU.add, op1=ALU.add,
            )
        nc.sync.dma_start(out=yr[:, m:m + 1, :], in_=ybig[:, m:m + 1, :])


# DVE self-waits are load-bearing for the CoreSim race detector (engine
# FIFO order alone is not credited), so never strip them.
ENGINE_SEM_PREFIX = {
    "PE": "PE_",
    "Activation": "Activation_",
    "Pool": "Pool_",
    "SP": "SP_",
}


def _strip_self_waits(nc):
    """Drop same-engine semaphore self-waits from multi-wait instructions.

    Engines execute and complete their own instructions in program order,
    so a wait on the engine's own completion semaphore is redundant whenever
    the instruction carries another wait — and walrus's PE/ACT instruction
    structs only encode a single wait.
    """
    n = 0
    for inst in nc.inst_map.values():
        si = getattr(inst, "sync_info", None)
        if si is None or not si.on_wait or len(si.on_wait) <= 1:
            continue
        eng = str(getattr(inst, "engine", "")).split(".")[-1]
        pref = ENGINE_SEM_PREFIX.get(eng)
        if pref is None:
            continue
        keep = [w for w in si.on_wait if not w.ant_name.startswith(pref)]
        if len(keep) != len(si.on_wait) and keep:
            inst.sync_info = mybir.SyncInfo(
                on_wait=keep, on_update=list(si.on_update or [])
            )
            n += 1
    return n


def build_nc():
    _install_drain_split()
    nc = bass.Bass(trn_type="TRN2", debug=False, num_devices=8)
    x_d = nc.dram_tensor("xb", [C, S], BF16, kind="ExternalInput")
    w_d = nc.dram_tensor("wallb", [C, NWCOL], BF16, kind="ExternalInput")
    b_d = nc.dram_tensor("bof", [C, 1], F32, kind="ExternalInput")
    y_d = nc.dram_tensor("y", [C, S], BF16, kind="ExternalOutput")
    with tile.TileContext(nc) as tc, ExitStack() as ctx:
        trace_kernel(ctx, tc, nc, x_d.ap(), w_d.ap(), b_d.ap(), y_d.ap())
    _strip_self_waits(nc)
    if not nc.is_finalized():
        nc.finalize()
    return nc


def host_inputs(x, Wqkv, Wo, bo):
    """Host-side reshard: per-core input dicts (weights replicated)."""
    x = np.asarray(x, dtype=np.float32)
    Wqkv = np.asarray(Wqkv, dtype=np.float32)
    Wo = np.asarray(Wo, dtype=np.float32)
    bo = np.asarray(bo, dtype=np.float32)

    # Wqkv rows per head h: [h*3D, h*3D+D) = q, [+D, +2D) = k, [+2D, +3D) = v.
    # q,k channel order: per pair -> [q(2p)|q(2p+1)], [k(2p)|k(2p+1)] tiles.
    order = []
    for p in range(NPAIR):
        for h in (2 * p, 2 * p + 1):
            order.extend(range(h * 3 * D, h * 3 * D + D))          # q rows
        for h in (2 * p, 2 * p + 1):
            order.extend(range(h * 3 * D + D, h * 3 * D + 2 * D))  # k rows
    wqkt = Wqkv[order].T                                            # (C, 2C)
    v_order = [h * 3 * D + 2 * D + d for h in range(NH) for d in range(D)]
    wvt = Wqkv[v_order].T                                           # (C, C)
    wot = Wo.T                                                      # (C, C)
    ident = np.zeros((C, P), dtype=np.float32)
    ident[0:P, 0:P] = np.eye(P)
    wallb = np.ascontiguousarray(
        np.concatenate([wqkt, wvt, wot, ident], axis=1)
    ).astype(ml_dtypes.bfloat16)                                    # (C, 4C+128)
    bof = np.ascontiguousarray(bo[:, None])                         # (C, 1)

    xb = x.reshape(B, C, S).astype(ml_dtypes.bfloat16)
    return [
        dict(xb=np.ascontiguousarray(xb[b]), wallb=wallb, bof=bof)
        for b in range(B)
    ]


_NC_CACHE = []

try:
    # bass_exec HLO does not embed the BIR; bust jax's executable cache so a
    # rebuilt kernel is actually recompiled instead of hitting a stale NEFF.
    import jax as _jax

    _jax.clear_caches()
except Exception:
    pass


def get_nc():
    if not _NC_CACHE:
        _NC_CACHE.append(build_nc())
    return _NC_CACHE[0]


def run(in_maps, **kwargs):
    return run_bass_kernel_spmd(get_nc(), in_maps, core_ids=list(range(B)), **kwargs)


def kernel(x, Wqkv, Wo, bo):
    in_maps = host_inputs(x, Wqkv, Wo, bo)
    r = run(in_maps)
    yv = np.stack([r.results[b]["y"].reshape(C, H, W) for b in range(B)])
    return yv.astype(np.float32)


if __name__ == "__main__":
    nc = build_nc()
    print("built ok:", len(nc.inst_map), "instructions")
